# revision 27
# baseline (speedup 1.0000x reference)
"""Bahdanau attention Trainium2 kernel.

Problem shapes: query [64, 512], values [64, 2048, 512],
W1/W2 [512, 256], b1/b2 [256], V [256, 1], bV [1].
Returns (context [64, 512], attn [64, 2048, 1]).

Sharding: data-parallel over batch across 8 NeuronCores (8 examples per
core); W1/W2/V replicated. All reductions are per-example, no cross-core
communication.

Two kernel modes:

"bf16x" (default): values are host-cast to bf16 and the transposed copy
  needed for the d-contraction (PE matmuls contract the partition dim) is
  produced by DMA xbar transposes straight from DRAM - the PE does *no*
  transpose-mode work, which measured ~0.5us per 128x128 tile on HW (it
  neither pipelines nor engages the HAM clock un-throttle). Scores are
  computed into a [128s, 16] transposed layout (th chunk stationary,
  V streaming) so the softmax is partition-parallel and the attention row
  needs only one tiny transpose back. The e^x sum is broadcast across
  partitions with a ones-matmul. Matmul inputs are bf16 (fp32 PSUM
  accumulation).

"f32r": fp32 data everywhere, PE in single-pass float32r mode, values
  transposed on the PE. ~10x better relative error, ~2.5x slower.
"""

import numpy as np

B, S, D, U = 64, 2048, 512, 256
N_CORES = 8
N_EX = B // N_CORES  # examples per core
P = 128
N_T = S // P        # 16 s-chunks per example
N_DC = D // P       # 4 d-chunks
N_UB = U // P       # 2 u-chunks
N_SB = S // 512     # 4 s-blocks of 512
MM_MODE = "bf16t"   # "bf16t" | "bf16x" | "f32r" | "f32"

_CACHE = {}


def _split_multiwait(nc, max_waits=1):
    """The container's walrus rejects >1 sync-wait on one instruction
    (setupSyncWait: 'Too many sync wait commands' on the TileContext final
    drain). Hoist extra waits onto single-wait drains placed just before."""
    import concourse.mybir as mybir

    for f in nc.m.functions:
        for bb in f.blocks:
            insts = bb.instructions
            patched = []
            changed = False
            for inst in insts:
                si = inst.sync_info
                if si is not None and si.on_wait and len(si.on_wait) > max_waits:
                    waits = list(si.on_wait)
                    for k, w in enumerate(waits[:-max_waits]):
                        d = mybir.InstDrain(name=f"{inst.name}-sw{k}", ins=[], outs=[])
                        d.engine = inst.engine
                        d.sync_info = mybir.SyncInfo(on_wait=[w], on_update=[])
                        patched.append(d)
                    inst.sync_info = mybir.SyncInfo(
                        on_wait=waits[-max_waits:], on_update=list(si.on_update)
                    )
                    changed = True
                patched.append(inst)
            if changed:
                insts[:] = patched
                if len(bb.instructions) != len(patched):
                    raise RuntimeError("failed to patch block instructions")


def build_nc(n_ex=N_EX, mm_mode=MM_MODE, split_waits=True, reps=1,
             pvt_bufs=4, ppv_bufs=3, psc_bufs=2, evac_mod=2):
    import concourse.bass as bass
    import concourse.mybir as mybir
    import concourse.tile as tile
    from concourse.masks import make_identity

    f32 = mybir.dt.float32
    bf16 = mybir.dt.bfloat16
    if mm_mode in ("bf16x", "bf16t"):
        mdt = bf16
    elif mm_mode == "f32r":
        mdt = mybir.dt.float32r
    else:
        mdt = f32
    AF = mybir.ActivationFunctionType

    def mb(ap):
        if mm_mode == "f32r" and ap.dtype != mdt:
            return ap.bitcast(mdt)
        return ap

    nc = bass.Bass()
    q_d = nc.declare_dram_parameter("query", [n_ex, D], f32, isOutput=False)
    v_d = vb_d = vbt_d = None
    if mm_mode in ("bf16x", "bf16t"):
        vb_d = nc.declare_dram_parameter("values_bf16", [n_ex, S, D], bf16,
                                         isOutput=False)
        if mm_mode == "bf16t":
            vbt_d = nc.declare_dram_parameter("values_bf16_T", [n_ex, D, S], bf16,
                                              isOutput=False)
    else:
        v_d = nc.declare_dram_parameter("values", [n_ex, S, D], f32, isOutput=False)
    w1_d = nc.declare_dram_parameter("W1", [D, U], f32, isOutput=False)
    b1_d = nc.declare_dram_parameter("b1", [U], f32, isOutput=False)
    w2_d = nc.declare_dram_parameter("W2", [D, U], f32, isOutput=False)
    b2_d = nc.declare_dram_parameter("b2", [U], f32, isOutput=False)
    vv_d = nc.declare_dram_parameter("V", [U, 1], f32, isOutput=False)
    # bV shifts every score equally; softmax is shift-invariant so it is
    # mathematically irrelevant, but declare it so input maps line up.
    bv_d = nc.declare_dram_parameter("bV", [1], f32, isOutput=False)
    ctx_d = nc.declare_dram_parameter("context", [n_ex, D], f32, isOutput=True)
    attn_d = nc.declare_dram_parameter("attn", [n_ex, S], f32, isOutput=True)

    env = dict(
        n_ex=n_ex, mm_mode=mm_mode, mdt=mdt, mb=mb,
        pvt_bufs=pvt_bufs, ppv_bufs=ppv_bufs, psc_bufs=psc_bufs,
        evac_mod=evac_mod,
        q_d=q_d, v_d=v_d, vb_d=vb_d, vbt_d=vbt_d, w1_d=w1_d, b1_d=b1_d, w2_d=w2_d,
        b2_d=b2_d, vv_d=vv_d, ctx_d=ctx_d, attn_d=attn_d,
    )
    with tile.TileContext(nc) as tc:
        if mm_mode in ("bf16x", "bf16t"):
            _body_bf16x(nc, tc, mybir, make_identity, env, reps)
        else:
            _body_f32r(nc, tc, mybir, make_identity, env, reps)

    if split_waits:
        _split_multiwait(nc)
    return nc


def _body_bf16x(nc, tc, mybir, make_identity, env, reps):
    f32 = mybir.dt.float32
    bf16 = mybir.dt.bfloat16
    AF = mybir.ActivationFunctionType
    n_ex = env["n_ex"]
    vbt_d = env["vbt_d"]
    q_d, vb_d, w1_d, b1_d, w2_d, b2_d, vv_d = (
        env["q_d"], env["vb_d"], env["w1_d"], env["b1_d"], env["w2_d"],
        env["b2_d"], env["vv_d"])
    ctx_d, attn_d = env["ctx_d"], env["attn_d"]

    with (
        tc.tile_pool(name="consts", bufs=1) as consts,
        tc.tile_pool(name="vpool", bufs=3) as vpool,
        tc.tile_pool(name="vtpool", bufs=3) as vtpool,
        tc.tile_pool(name="thpool", bufs=2) as thpool,
        tc.tile_pool(name="expool", bufs=3) as expool,
        tc.tile_pool(name="atpool", bufs=3) as atpool,
        tc.tile_pool(name="outpool", bufs=3) as outpool,
        tc.tile_pool(name="smallpool", bufs=3) as smallpool,
        tc.tile_pool(name="ppv_pool", bufs=3, space="PSUM") as ppv_pool,
        tc.tile_pool(name="psct_pool", bufs=2, space="PSUM") as psct_pool,
        tc.tile_pool(name="pat_pool", bufs=1, space="PSUM") as pat_pool,
        tc.tile_pool(name="pctx_pool", bufs=1, space="PSUM") as pctx_pool,
        tc.tile_pool(name="psum_pool", bufs=1, space="PSUM") as psum_pool,
    ):
        # ---- one-time setup ------------------------------------------------
        identity_f32 = consts.tile([P, P], f32)
        make_identity(nc, identity_f32)

        ones_sb = consts.tile([P, P], f32)
        nc.vector.memset(ones_sb[:], 1.0)

        w1_f = consts.tile([P, N_DC, U], f32)
        nc.sync.dma_start(w1_f[:], w1_d.rearrange("(c p) u -> p c u", p=P))
        w1_sb = consts.tile([P, N_DC, U], bf16)
        nc.vector.tensor_copy(w1_sb[:], w1_f[:])
        w2_sb = consts.tile([P, N_DC, U], f32)
        nc.sync.dma_start(w2_sb[:], w2_d.rearrange("(c p) u -> p c u", p=P))
        v_f = consts.tile([P, N_UB], f32)
        nc.sync.dma_start(v_f[:], vv_d.rearrange("(c p) o -> p (c o)", p=P))
        v_sb = consts.tile([P, N_UB], bf16)
        nc.vector.tensor_copy(v_sb[:], v_f[:])
        b1_sb = consts.tile([P, N_UB], f32)
        nc.sync.dma_start(b1_sb[:], b1_d.rearrange("(c p) -> p c", p=P))
        b2_sb = consts.tile([P, N_UB], f32)
        nc.sync.dma_start(b2_sb[:], b2_d.rearrange("(c p) -> p c", p=P))
        q_sb = consts.tile([n_ex, D], f32)
        nc.sync.dma_start(q_sb[:], q_d[:])

        b12_sb = consts.tile([P, N_UB], f32)
        nc.vector.tensor_add(b12_sb[:], b1_sb[:], b2_sb[:])

        # constant softmax shift: |score| <= ||V||_1 ~ 10, so a fixed shift
        # replaces the max subtraction (softmax is shift-invariant)
        negshift = consts.tile([P, 1], f32)
        nc.vector.memset(negshift[:], -12.0)

        # query^T via PE transposes (fp32, tiny) then pq = W2^T q + b1 + b2
        qt_sb = consts.tile([P, N_DC, n_ex], f32)
        for dc in range(N_DC):
            pqt = pat_pool.tile([P, n_ex], f32, tag="pat", name="pqt")
            nc.tensor.transpose(
                pqt[:], q_sb[:, dc * P : (dc + 1) * P], identity_f32[:n_ex, :n_ex]
            )
            nc.vector.tensor_copy(qt_sb[:, dc, :], pqt[:])
        pqb_sb = consts.tile([P, N_UB, n_ex], f32)
        for ub in range(N_UB):
            ppq = ppv_pool.tile([P, n_ex], f32, tag="ppv", name="ppq")
            for dc in range(N_DC):
                nc.tensor.matmul(
                    ppq[:],
                    w2_sb[:, dc, ub * P : (ub + 1) * P],
                    qt_sb[:, dc, :],
                    start=(dc == 0),
                    stop=(dc == N_DC - 1),
                )
            nc.vector.tensor_scalar_add(
                pqb_sb[:, ub, :], ppq[:], b12_sb[:, ub : ub + 1]
            )

        # ---- per-example pipeline ------------------------------------------
        for e in [e for _ in range(reps) for e in range(n_ex)]:
            # natural layout (for the context matmul): [128s, t, d]
            v_nat = vpool.tile([P, N_T, D], bf16)
            for t in range(N_T):
                nc.sync.dma_start(v_nat[:, t, :], vb_d[e, t * P : (t + 1) * P, :])

            # transposed layout: plain loads from the host-pretransposed
            # copy when available, else DMA xbar straight from DRAM
            vt = vtpool.tile([P, N_DC, S], bf16)
            if vbt_d is not None:
                for dc in range(N_DC):
                    nc.sync.dma_start(
                        vt[:, dc, :], vbt_d[e, dc * P : (dc + 1) * P, :]
                    )
            else:
                for dc in range(N_DC):
                    nc.sync.dma_start(
                        vt[:, dc, :],
                        vb_d[e, :, dc * P : (dc + 1) * P],
                        transpose=True,
                    )

            # pv^T = W1^T @ v^T; tanh(+bias) fused on ACT -> th [128u, ub, s]
            th = thpool.tile([P, N_UB, S], bf16)
            for ub in range(N_UB):
                for sb in range(N_SB):
                    ppv = ppv_pool.tile([P, 512], f32, tag="ppv")
                    for dc in range(N_DC):
                        nc.tensor.matmul(
                            ppv[:],
                            w1_sb[:, dc, ub * P : (ub + 1) * P],
                            vt[:, dc, sb * 512 : (sb + 1) * 512],
                            start=(dc == 0),
                            stop=(dc == N_DC - 1),
                        )
                    nc.scalar.activation(
                        th[:, ub, sb * 512 : (sb + 1) * 512],
                        ppv[:],
                        AF.Tanh,
                        bias=pqb_sb[:, ub, e : e + 1],
                        scale=1.0,
                    )

            # score^T [128s, 16t]: th chunk stationary, V streams (N=1)
            psct = psct_pool.tile([P, N_T], f32, tag="psct")
            for t in range(N_T):
                for ub in range(N_UB):
                    nc.tensor.matmul(
                        psct[:, t : t + 1],
                        th[:, ub, t * P : (t + 1) * P],
                        v_sb[:, ub : ub + 1],
                        start=(ub == 0),
                        stop=(ub == N_UB - 1),
                    )

            # partition-parallel softmax on [128, 16]
            expT = expool.tile([P, N_T], f32, name="expT")
            sums = smallpool.tile([P, 1], f32, tag="sums")
            nc.scalar.activation(
                expT[:], psct[:], AF.Exp, bias=negshift[:], scale=1.0,
                accum_out=sums[:],
            )
            # total = ones^T @ per-partition sums, broadcast to all partitions
            psum_b = psum_pool.tile([P, 1], f32, tag="psum_b")
            nc.tensor.matmul(psum_b[:], ones_sb[:], sums[:], start=True, stop=True)
            recip_b = smallpool.tile([P, 1], f32, tag="recip_b")
            nc.vector.reciprocal(recip_b[:], psum_b[:])

            # context = (exp^T @ values) * (1/sum); lhsT columns in bf16
            at_sb = atpool.tile([P, N_T], bf16)
            nc.vector.tensor_copy(at_sb[:], expT[:])
            pctx = pctx_pool.tile([1, D], f32, tag="pctx")
            for t in range(N_T):
                nc.tensor.matmul(
                    pctx[:],
                    at_sb[:, t : t + 1],
                    v_nat[:, t, :],
                    start=(t == 0),
                    stop=(t == N_T - 1),
                )
            ctx_row = outpool.tile([1, D], f32, tag="ctx_row")
            nc.vector.tensor_scalar_mul(ctx_row[:], pctx[:], recip_b[0:1, :])
            nc.sync.dma_start(ctx_d[e : e + 1, :], ctx_row[:])

            # attn output: scale, transpose back on PE (one op), write out
            asc = expool.tile([P, N_T], f32, name="asc")
            nc.vector.tensor_scalar_mul(asc[:], expT[:], recip_b[:])
            pat = pat_pool.tile([N_T, P], f32, tag="pat", name="pat_at")
            nc.tensor.transpose(pat[:], asc[:], identity_f32[:])
            attn_sb = outpool.tile([N_T, P], f32, tag="attn_sb")
            nc.scalar.copy(attn_sb[:], pat[:])
            nc.sync.dma_start(
                attn_d[e].rearrange("(t p) -> t p", p=P), attn_sb[:]
            )


def _body_f32r(nc, tc, mybir, make_identity, env, reps):
    f32 = mybir.dt.float32
    AF = mybir.ActivationFunctionType
    mm_mode = env["mm_mode"]
    mdt = env["mdt"]
    mb = env["mb"]
    n_ex = env["n_ex"]
    pvt_bufs, ppv_bufs, psc_bufs, evac_mod = (
        env["pvt_bufs"], env["ppv_bufs"], env["psc_bufs"], env["evac_mod"])
    q_d, v_d, w1_d, b1_d, w2_d, b2_d, vv_d = (
        env["q_d"], env["v_d"], env["w1_d"], env["b1_d"], env["w2_d"],
        env["b2_d"], env["vv_d"])
    ctx_d, attn_d = env["ctx_d"], env["attn_d"]

    with (
        tc.tile_pool(name="consts", bufs=1) as consts,
        tc.tile_pool(name="vpool", bufs=2) as vpool,
        tc.tile_pool(name="vtpool", bufs=2) as vtpool,
        tc.tile_pool(name="thpool", bufs=2) as thpool,
        tc.tile_pool(name="rowpool", bufs=3) as rowpool,
        tc.tile_pool(name="atpool", bufs=2) as atpool,
        tc.tile_pool(name="ctxpool", bufs=2) as ctxpool,
        tc.tile_pool(name="smallpool", bufs=2) as smallpool,
        tc.tile_pool(name="pvt_pool", bufs=pvt_bufs, space="PSUM") as pvt_pool,
        tc.tile_pool(name="ppv_pool", bufs=ppv_bufs, space="PSUM") as ppv_pool,
        tc.tile_pool(name="psc_pool", bufs=psc_bufs, space="PSUM") as psc_pool,
        tc.tile_pool(name="pctx_pool", bufs=1, space="PSUM") as pctx_pool,
    ):
        identity_f32 = consts.tile([P, P], f32)
        make_identity(nc, identity_f32)
        if mm_mode == "f32r":
            identity = consts.tile([P, P], mdt)
            nc.vector.tensor_copy(identity[:], identity_f32[:])
        else:
            identity = identity_f32

        w1_sb = consts.tile([P, N_DC, U], mdt)
        nc.sync.dma_start(w1_sb[:], mb(w1_d.rearrange("(c p) u -> p c u", p=P)))
        w2_sb = consts.tile([P, N_DC, U], mdt)
        nc.sync.dma_start(w2_sb[:], mb(w2_d.rearrange("(c p) u -> p c u", p=P)))
        v_sb = consts.tile([P, N_UB], mdt)
        nc.sync.dma_start(v_sb[:], mb(vv_d.rearrange("(c p) o -> p (c o)", p=P)))
        b1_sb = consts.tile([P, N_UB], f32)
        nc.sync.dma_start(b1_sb[:], b1_d.rearrange("(c p) -> p c", p=P))
        b2_sb = consts.tile([P, N_UB], f32)
        nc.sync.dma_start(b2_sb[:], b2_d.rearrange("(c p) -> p c", p=P))
        q_sb = consts.tile([n_ex, D], mdt)
        nc.sync.dma_start(q_sb[:], mb(q_d[:]))

        b12_sb = consts.tile([P, N_UB], f32)
        nc.vector.tensor_add(b12_sb[:], b1_sb[:], b2_sb[:])

        negshift = consts.tile([1, 1], f32)
        nc.vector.memset(negshift[:], -12.0)

        qt_sb = consts.tile([P, N_DC, n_ex], mdt)
        for dc in range(N_DC):
            pqt = pvt_pool.tile([P, n_ex], mdt, tag="pvt", name="pqt")
            nc.tensor.transpose(
                pqt[:], q_sb[:, dc * P : (dc + 1) * P], mb(identity[:n_ex, :n_ex])
            )
            nc.vector.tensor_copy(qt_sb[:, dc, :], pqt[:])

        pqb_sb = consts.tile([P, N_UB, n_ex], f32)
        for ub in range(N_UB):
            ppq = ppv_pool.tile([P, n_ex], f32, tag="ppv", name="ppq")
            for dc in range(N_DC):
                nc.tensor.matmul(
                    ppq[:],
                    w2_sb[:, dc, ub * P : (ub + 1) * P],
                    qt_sb[:, dc, :],
                    start=(dc == 0),
                    stop=(dc == N_DC - 1),
                )
            nc.vector.tensor_scalar_add(
                pqb_sb[:, ub, :], ppq[:], b12_sb[:, ub : ub + 1]
            )

        for e in [e for _ in range(reps) for e in range(n_ex)]:
            v_nat = vpool.tile([P, N_T, D], mdt)
            for t in range(N_T):
                nc.sync.dma_start(
                    v_nat[:, t, :], mb(v_d[e, t * P : (t + 1) * P, :])
                )

            vt = vtpool.tile([P, N_DC, S], mdt)
            for t in range(N_T):
                pvt = pvt_pool.tile([P, N_DC, P], mdt, tag="pvt")
                for dc in range(N_DC):
                    nc.tensor.transpose(
                        pvt[:, dc, :],
                        v_nat[:, t, dc * P : (dc + 1) * P],
                        mb(identity[:]),
                    )
                if t % evac_mod != evac_mod - 1:
                    nc.vector.tensor_copy(vt[:, :, t * P : (t + 1) * P], pvt[:])
                else:
                    nc.scalar.copy(vt[:, :, t * P : (t + 1) * P], pvt[:])

            th = thpool.tile([P, N_UB, S], mdt)
            for ub in range(N_UB):
                for sb in range(N_SB):
                    ppv = ppv_pool.tile([P, 512], f32, tag="ppv")
                    for dc in range(N_DC):
                        nc.tensor.matmul(
                            ppv[:],
                            w1_sb[:, dc, ub * P : (ub + 1) * P],
                            vt[:, dc, sb * 512 : (sb + 1) * 512],
                            start=(dc == 0),
                            stop=(dc == N_DC - 1),
                        )
                    nc.scalar.activation(
                        th[:, ub, sb * 512 : (sb + 1) * 512],
                        ppv[:],
                        AF.Tanh,
                        bias=pqb_sb[:, ub, e : e + 1],
                        scale=1.0,
                    )

            score_row = rowpool.tile([1, S], f32, tag="row", name="score_row")
            for sb in range(N_SB):
                psc = psc_pool.tile([1, 512], f32, tag="psc")
                for ub in range(N_UB):
                    nc.tensor.matmul(
                        psc[:],
                        v_sb[:, ub : ub + 1],
                        th[:, ub, sb * 512 : (sb + 1) * 512],
                        start=(ub == 0),
                        stop=(ub == N_UB - 1),
                    )
                nc.vector.tensor_copy(score_row[:, sb * 512 : (sb + 1) * 512], psc[:])

            exp_row = rowpool.tile([1, S], f32, tag="row", name="exp_row")
            sumexp = smallpool.tile([1, 1], f32, tag="sumexp")
            nc.scalar.activation(
                exp_row[:], score_row[:], AF.Exp, bias=negshift[:], scale=1.0,
                accum_out=sumexp[:],
            )
            recip = smallpool.tile([1, 1], f32, tag="recip")
            nc.vector.reciprocal(recip[:], sumexp[:])
            attn_row = rowpool.tile([1, S], f32, tag="row", name="attn_row")
            nc.gpsimd.tensor_scalar_mul(attn_row[:], exp_row[:], recip[:])
            nc.sync.dma_start(attn_d[e : e + 1, :], attn_row[:])

            pat = psc_pool.tile([P, N_T], f32, tag="psc", name="pat")
            for t in range(N_T):
                nc.tensor.transpose(
                    pat[:, t : t + 1],
                    exp_row[:, t * P : (t + 1) * P],
                    identity_f32[:1, :1],
                )
            at_sb = atpool.tile([P, N_T], mdt)
            nc.vector.tensor_copy(at_sb[:], pat[:])

            pctx = pctx_pool.tile([1, D], f32, tag="pctx")
            for t in range(N_T):
                nc.tensor.matmul(
                    pctx[:],
                    at_sb[:, t : t + 1],
                    v_nat[:, t, :],
                    start=(t == 0),
                    stop=(t == N_T - 1),
                )
            ctx_row = ctxpool.tile([1, D], f32, tag="ctx_row")
            nc.vector.tensor_scalar_mul(ctx_row[:], pctx[:], recip[:])
            nc.sync.dma_start(ctx_d[e : e + 1, :], ctx_row[:])


def kernel(query, values, W1, b1, W2, b2, V, bV):
    import ml_dtypes
    from concourse.bass_utils import run_bass_kernel_spmd

    key = ("nc", N_EX, MM_MODE)
    if key not in _CACHE:
        _CACHE[key] = build_nc(N_EX, MM_MODE)
    nc = _CACHE[key]

    query = np.asarray(query, dtype=np.float32)
    values = np.asarray(values, dtype=np.float32)
    shared = {
        "W1": np.asarray(W1, dtype=np.float32),
        "b1": np.asarray(b1, dtype=np.float32),
        "W2": np.asarray(W2, dtype=np.float32),
        "b2": np.asarray(b2, dtype=np.float32),
        "V": np.asarray(V, dtype=np.float32),
        "bV": np.asarray(bV, dtype=np.float32),
    }
    in_maps = []
    for c in range(N_CORES):
        lo, hi = c * N_EX, (c + 1) * N_EX
        m = {"query": query[lo:hi], **shared}
        if MM_MODE in ("bf16x", "bf16t"):
            vb = values[lo:hi].astype(ml_dtypes.bfloat16)
            m["values_bf16"] = vb
            if MM_MODE == "bf16t":
                m["values_bf16_T"] = np.ascontiguousarray(vb.transpose(0, 2, 1))
        else:
            m["values"] = values[lo:hi]
        in_maps.append(m)

    res = run_bass_kernel_spmd(nc, in_maps, list(range(N_CORES)))

    context = np.concatenate([res.results[c]["context"] for c in range(N_CORES)], axis=0)
    attn = np.concatenate([res.results[c]["attn"] for c in range(N_CORES)], axis=0)
    return context, attn[:, :, None]


# revision 29
# speedup vs baseline: 1.1694x; 1.1694x over previous
"""Bahdanau attention Trainium2 kernel.

Problem shapes: query [64, 512], values [64, 2048, 512],
W1/W2 [512, 256], b1/b2 [256], V [256, 1], bV [1].
Returns (context [64, 512], attn [64, 2048, 1]).

Sharding: data-parallel over batch across 8 NeuronCores (8 examples per
core); W1/W2/V replicated. All reductions are per-example, no cross-core
communication.

Two kernel modes:

"bf16x" (default): values are host-cast to bf16 and the transposed copy
  needed for the d-contraction (PE matmuls contract the partition dim) is
  produced by DMA xbar transposes straight from DRAM - the PE does *no*
  transpose-mode work, which measured ~0.5us per 128x128 tile on HW (it
  neither pipelines nor engages the HAM clock un-throttle). Scores are
  computed into a [128s, 16] transposed layout (th chunk stationary,
  V streaming) so the softmax is partition-parallel and the attention row
  needs only one tiny transpose back. The e^x sum is broadcast across
  partitions with a ones-matmul. Matmul inputs are bf16 (fp32 PSUM
  accumulation).

"f32r": fp32 data everywhere, PE in single-pass float32r mode, values
  transposed on the PE. ~10x better relative error, ~2.5x slower.
"""

import numpy as np

B, S, D, U = 64, 2048, 512, 256
N_CORES = 8
N_EX = B // N_CORES  # examples per core
P = 128
N_T = S // P        # 16 s-chunks per example
N_DC = D // P       # 4 d-chunks
N_UB = U // P       # 2 u-chunks
N_SB = S // 512     # 4 s-blocks of 512
MM_MODE = "bf16t"   # "bf16t" | "bf16x" | "f32r" | "f32"

_CACHE = {}


def _split_multiwait(nc, max_waits=1):
    """The container's walrus rejects >1 sync-wait on one instruction
    (setupSyncWait: 'Too many sync wait commands' on the TileContext final
    drain). Hoist extra waits onto single-wait drains placed just before."""
    import concourse.mybir as mybir

    for f in nc.m.functions:
        for bb in f.blocks:
            insts = bb.instructions
            patched = []
            changed = False
            for inst in insts:
                si = inst.sync_info
                if si is not None and si.on_wait and len(si.on_wait) > max_waits:
                    waits = list(si.on_wait)
                    for k, w in enumerate(waits[:-max_waits]):
                        d = mybir.InstDrain(name=f"{inst.name}-sw{k}", ins=[], outs=[])
                        d.engine = inst.engine
                        d.sync_info = mybir.SyncInfo(on_wait=[w], on_update=[])
                        patched.append(d)
                    inst.sync_info = mybir.SyncInfo(
                        on_wait=waits[-max_waits:], on_update=list(si.on_update)
                    )
                    changed = True
                patched.append(inst)
            if changed:
                insts[:] = patched
                if len(bb.instructions) != len(patched):
                    raise RuntimeError("failed to patch block instructions")


def build_nc(n_ex=N_EX, mm_mode=MM_MODE, split_waits=True, reps=1,
             pvt_bufs=4, ppv_bufs=3, psc_bufs=2, evac_mod=2, nat=True,
             dma_split=True, nat_merge=4):
    import concourse.bass as bass
    import concourse.mybir as mybir
    import concourse.tile as tile
    from concourse.masks import make_identity

    f32 = mybir.dt.float32
    bf16 = mybir.dt.bfloat16
    if mm_mode in ("bf16x", "bf16t"):
        mdt = bf16
    elif mm_mode == "f32r":
        mdt = mybir.dt.float32r
    else:
        mdt = f32
    AF = mybir.ActivationFunctionType

    def mb(ap):
        if mm_mode == "f32r" and ap.dtype != mdt:
            return ap.bitcast(mdt)
        return ap

    nc = bass.Bass()
    q_d = nc.declare_dram_parameter("query", [n_ex, D], f32, isOutput=False)
    v_d = vb_d = vbt_d = None
    if mm_mode in ("bf16x", "bf16t"):
        vb_d = nc.declare_dram_parameter("values_bf16", [n_ex, S, D], bf16,
                                         isOutput=False)
        if mm_mode == "bf16t":
            vbt_d = nc.declare_dram_parameter("values_bf16_T", [n_ex, D, S], bf16,
                                              isOutput=False)
    else:
        v_d = nc.declare_dram_parameter("values", [n_ex, S, D], f32, isOutput=False)
    w1_d = nc.declare_dram_parameter("W1", [D, U], f32, isOutput=False)
    b1_d = nc.declare_dram_parameter("b1", [U], f32, isOutput=False)
    w2_d = nc.declare_dram_parameter("W2", [D, U], f32, isOutput=False)
    b2_d = nc.declare_dram_parameter("b2", [U], f32, isOutput=False)
    vv_d = nc.declare_dram_parameter("V", [U, 1], f32, isOutput=False)
    # bV shifts every score equally; softmax is shift-invariant so it is
    # mathematically irrelevant, but declare it so input maps line up.
    bv_d = nc.declare_dram_parameter("bV", [1], f32, isOutput=False)
    ctx_d = nc.declare_dram_parameter("context", [n_ex, D], f32, isOutput=True)
    attn_d = nc.declare_dram_parameter("attn", [n_ex, S], f32, isOutput=True)

    env = dict(
        n_ex=n_ex, mm_mode=mm_mode, mdt=mdt, mb=mb,
        pvt_bufs=pvt_bufs, ppv_bufs=ppv_bufs, psc_bufs=psc_bufs,
        evac_mod=evac_mod,
        q_d=q_d, v_d=v_d, vb_d=vb_d, vbt_d=vbt_d, w1_d=w1_d, b1_d=b1_d, w2_d=w2_d,
        b2_d=b2_d, vv_d=vv_d, ctx_d=ctx_d, attn_d=attn_d, nat=nat,
        dma_split=dma_split, nat_merge=nat_merge,
    )
    with tile.TileContext(nc) as tc:
        if mm_mode in ("bf16x", "bf16t"):
            _body_bf16x(nc, tc, mybir, make_identity, env, reps)
        else:
            _body_f32r(nc, tc, mybir, make_identity, env, reps)

    if split_waits:
        _split_multiwait(nc)
    return nc


def _body_bf16x(nc, tc, mybir, make_identity, env, reps):
    from concourse.bass import AP as bass_AP
    f32 = mybir.dt.float32
    bf16 = mybir.dt.bfloat16
    AF = mybir.ActivationFunctionType
    n_ex = env["n_ex"]
    vbt_d = env["vbt_d"]
    q_d, vb_d, w1_d, b1_d, w2_d, b2_d, vv_d = (
        env["q_d"], env["vb_d"], env["w1_d"], env["b1_d"], env["w2_d"],
        env["b2_d"], env["vv_d"])
    ctx_d, attn_d = env["ctx_d"], env["attn_d"]
    nat = env["nat"]
    dma_split = env["dma_split"]
    nat_merge = env["nat_merge"]
    if not nat:
        exp_bounce = nc.dram_tensor("exp_bounce", [n_ex, S], bf16)

    with (
        tc.tile_pool(name="consts", bufs=1) as consts,
        tc.tile_pool(name="vpool", bufs=3) as vpool,
        tc.tile_pool(name="vtpool", bufs=3) as vtpool,
        tc.tile_pool(name="thpool", bufs=2) as thpool,
        tc.tile_pool(name="expool", bufs=3) as expool,
        tc.tile_pool(name="atpool", bufs=3) as atpool,
        tc.tile_pool(name="outpool", bufs=3) as outpool,
        tc.tile_pool(name="smallpool", bufs=3) as smallpool,
        tc.tile_pool(name="ppv_pool", bufs=3, space="PSUM") as ppv_pool,
        tc.tile_pool(name="psct_pool", bufs=2, space="PSUM") as psct_pool,
        tc.tile_pool(name="pat_pool", bufs=1, space="PSUM") as pat_pool,
        tc.tile_pool(name="pctx_pool", bufs=1, space="PSUM") as pctx_pool,
        tc.tile_pool(name="psum_pool", bufs=1, space="PSUM") as psum_pool,
    ):
        # ---- one-time setup ------------------------------------------------
        identity_f32 = consts.tile([P, P], f32)
        make_identity(nc, identity_f32)

        ones_sb = consts.tile([P, P], f32)
        nc.vector.memset(ones_sb[:], 1.0)

        w1_f = consts.tile([P, N_DC, U], f32)
        nc.sync.dma_start(w1_f[:], w1_d.rearrange("(c p) u -> p c u", p=P))
        w1_sb = consts.tile([P, N_DC, U], bf16)
        nc.vector.tensor_copy(w1_sb[:], w1_f[:])
        w2_sb = consts.tile([P, N_DC, U], f32)
        nc.sync.dma_start(w2_sb[:], w2_d.rearrange("(c p) u -> p c u", p=P))
        v_f = consts.tile([P, N_UB], f32)
        nc.sync.dma_start(v_f[:], vv_d.rearrange("(c p) o -> p (c o)", p=P))
        v_sb = consts.tile([P, N_UB], bf16)
        nc.vector.tensor_copy(v_sb[:], v_f[:])
        b1_sb = consts.tile([P, N_UB], f32)
        nc.sync.dma_start(b1_sb[:], b1_d.rearrange("(c p) -> p c", p=P))
        b2_sb = consts.tile([P, N_UB], f32)
        nc.sync.dma_start(b2_sb[:], b2_d.rearrange("(c p) -> p c", p=P))
        q_sb = consts.tile([n_ex, D], f32)
        nc.sync.dma_start(q_sb[:], q_d[:])

        b12_sb = consts.tile([P, N_UB], f32)
        nc.vector.tensor_add(b12_sb[:], b1_sb[:], b2_sb[:])

        # constant softmax shift: |score| <= ||V||_1 ~ 10, so a fixed shift
        # replaces the max subtraction (softmax is shift-invariant)
        negshift = consts.tile([P, 1], f32)
        nc.vector.memset(negshift[:], -12.0)

        # query^T via PE transposes (fp32, tiny) then pq = W2^T q + b1 + b2
        qt_sb = consts.tile([P, N_DC, n_ex], f32)
        for dc in range(N_DC):
            pqt = pat_pool.tile([P, n_ex], f32, tag="pat", name="pqt")
            nc.tensor.transpose(
                pqt[:], q_sb[:, dc * P : (dc + 1) * P], identity_f32[:n_ex, :n_ex]
            )
            nc.vector.tensor_copy(qt_sb[:, dc, :], pqt[:])
        pqb_sb = consts.tile([P, N_UB, n_ex], f32)
        for ub in range(N_UB):
            ppq = ppv_pool.tile([P, n_ex], f32, tag="ppv", name="ppq")
            for dc in range(N_DC):
                nc.tensor.matmul(
                    ppq[:],
                    w2_sb[:, dc, ub * P : (ub + 1) * P],
                    qt_sb[:, dc, :],
                    start=(dc == 0),
                    stop=(dc == N_DC - 1),
                )
            nc.vector.tensor_scalar_add(
                pqb_sb[:, ub, :], ppq[:], b12_sb[:, ub : ub + 1]
            )

        # ---- per-example pipeline ------------------------------------------
        for e in [e for _ in range(reps) for e in range(n_ex)]:
            # natural layout (for the context matmul): [128s, t, d]
            if nat:
                v_nat = vpool.tile([P, N_T, D], bf16)
                for t0 in range(0, N_T, nat_merge):
                    nc.sync.dma_start(
                        v_nat[:, t0 : t0 + nat_merge, :],
                        vb_d[e, t0 * P : (t0 + nat_merge) * P, :].rearrange(
                            "(q p) d -> p q d", p=P
                        ),
                    )

            # transposed layout: plain loads from the host-pretransposed
            # copy when available, else DMA xbar straight from DRAM
            vt = vtpool.tile([P, N_DC, S], bf16)
            if vbt_d is not None:
                vt_eng = nc.scalar if dma_split else nc.sync
                for dc in range(N_DC):
                    vt_eng.dma_start(
                        vt[:, dc, :], vbt_d[e, dc * P : (dc + 1) * P, :]
                    )
            else:
                for dc in range(N_DC):
                    nc.sync.dma_start(
                        vt[:, dc, :],
                        vb_d[e, :, dc * P : (dc + 1) * P],
                        transpose=True,
                    )

            # pv^T = W1^T @ v^T; tanh(+bias) fused on ACT -> th [128u, ub, s]
            th = thpool.tile([P, N_UB, S], bf16)
            for ub in range(N_UB):
                for sb in range(N_SB):
                    ppv = ppv_pool.tile([P, 512], f32, tag="ppv")
                    for dc in range(N_DC):
                        nc.tensor.matmul(
                            ppv[:],
                            w1_sb[:, dc, ub * P : (ub + 1) * P],
                            vt[:, dc, sb * 512 : (sb + 1) * 512],
                            start=(dc == 0),
                            stop=(dc == N_DC - 1),
                        )
                    nc.scalar.activation(
                        th[:, ub, sb * 512 : (sb + 1) * 512],
                        ppv[:],
                        AF.Tanh,
                        bias=pqb_sb[:, ub, e : e + 1],
                        scale=1.0,
                    )

            # score^T [128s, 16t]: th chunk stationary, V streams (N=1)
            psct = psct_pool.tile([P, N_T], f32, tag="psct")
            for t in range(N_T):
                for ub in range(N_UB):
                    nc.tensor.matmul(
                        psct[:, t : t + 1],
                        th[:, ub, t * P : (t + 1) * P],
                        v_sb[:, ub : ub + 1],
                        start=(ub == 0),
                        stop=(ub == N_UB - 1),
                    )

            # partition-parallel softmax on [128, 16]
            expT = expool.tile([P, N_T], f32, name="expT")
            sums = smallpool.tile([P, 1], f32, tag="sums")
            nc.scalar.activation(
                expT[:], psct[:], AF.Exp, bias=negshift[:], scale=1.0,
                accum_out=sums[:],
            )
            # total = ones^T @ per-partition sums, broadcast to all partitions
            psum_b = psum_pool.tile([P, 1], f32, tag="psum_b")
            nc.tensor.matmul(psum_b[:], ones_sb[:], sums[:], start=True, stop=True)
            recip_b = smallpool.tile([P, 1], f32, tag="recip_b")
            nc.vector.reciprocal(recip_b[:], psum_b[:])

            if nat:
                # context = (exp^T @ values) * (1/sum); lhsT columns in bf16
                at_sb = atpool.tile([P, N_T], bf16)
                nc.vector.tensor_copy(at_sb[:], expT[:])
                pctx = pctx_pool.tile([1, D], f32, tag="pctx")
                for t in range(N_T):
                    nc.tensor.matmul(
                        pctx[:],
                        at_sb[:, t : t + 1],
                        v_nat[:, t, :],
                        start=(t == 0),
                        stop=(t == N_T - 1),
                    )
                ctx_row = outpool.tile([1, D], f32, tag="ctx_row")
                nc.vector.tensor_scalar_mul(ctx_row[:], pctx[:], recip_b[0:1, :])
                nc.sync.dma_start(ctx_d[e : e + 1, :], ctx_row[:])

                # attn output: scale, transpose back on PE (one op), write out
                asc = expool.tile([P, N_T], f32, name="asc")
                nc.vector.tensor_scalar_mul(asc[:], expT[:], recip_b[:])
                pat = pat_pool.tile([N_T, P], f32, tag="pat", name="pat_at")
                nc.tensor.transpose(pat[:], asc[:], identity_f32[:])
                attn_sb = outpool.tile([N_T, P], f32, tag="attn_sb")
                nc.scalar.copy(attn_sb[:], pat[:])
                nc.sync.dma_start(
                    attn_d[e].rearrange("(t p) -> t p", p=P), attn_sb[:]
                )
            else:
                # one transpose of unscaled exp serves both outputs and the
                # broadcast row for the DVE context reduction
                pat = pat_pool.tile([N_T, P], f32, tag="pat", name="pat_at")
                nc.tensor.transpose(pat[:], expT[:], identity_f32[:])
                exp16_bf = outpool.tile([N_T, P], bf16, tag="exp16_bf")
                nc.scalar.copy(exp16_bf[:], pat[:])
                attn_sb = outpool.tile([N_T, P], f32, tag="attn_sb")
                nc.vector.tensor_scalar_mul(attn_sb[:], pat[:], recip_b[:N_T, :])
                nc.sync.dma_start(
                    attn_d[e].rearrange("(t p) -> t p", p=P), attn_sb[:]
                )
                nc.sync.dma_start(
                    exp_bounce[e].rearrange("(t p) -> t p", p=P), exp16_bf[:]
                )
                # broadcast exp row across all 128 partitions from DRAM
                attn_bc = vpool.tile([P, S], bf16, tag="attn_bc")
                row = exp_bounce[e]
                bc_ap = bass_AP(tensor=row.tensor, offset=row.offset,
                                ap=[[0, P]] + [list(x) for x in row.ap])
                nc.sync.dma_start(attn_bc[:], bc_ap)
                # ctx^T[d] = sum_s vt[d, s] * exp[s] on the DVE (fused reduce)
                scr = vpool.tile([P, S], bf16, tag="scr")
                ctxT = expool.tile([P, N_DC], f32, name="ctxT")
                for dc in range(N_DC):
                    nc.vector.tensor_tensor_reduce(
                        out=scr[:],
                        in0=vt[:, dc, :],
                        in1=attn_bc[:],
                        scale=1.0,
                        scalar=0.0,
                        op0=mybir.AluOpType.mult,
                        op1=mybir.AluOpType.add,
                        accum_out=ctxT[:, dc : dc + 1],
                    )
                pct = pat_pool.tile([N_DC, P], f32, tag="pat", name="pct")
                nc.tensor.transpose(pct[:], ctxT[:], identity_f32[:])
                ctx_row4 = outpool.tile([N_DC, P], f32, tag="ctx_row4")
                nc.vector.tensor_scalar_mul(ctx_row4[:], pct[:], recip_b[:N_DC, :])
                nc.sync.dma_start(
                    ctx_d[e].rearrange("(c p) -> c p", p=P), ctx_row4[:]
                )


def _body_f32r(nc, tc, mybir, make_identity, env, reps):
    f32 = mybir.dt.float32
    AF = mybir.ActivationFunctionType
    mm_mode = env["mm_mode"]
    mdt = env["mdt"]
    mb = env["mb"]
    n_ex = env["n_ex"]
    pvt_bufs, ppv_bufs, psc_bufs, evac_mod = (
        env["pvt_bufs"], env["ppv_bufs"], env["psc_bufs"], env["evac_mod"])
    q_d, v_d, w1_d, b1_d, w2_d, b2_d, vv_d = (
        env["q_d"], env["v_d"], env["w1_d"], env["b1_d"], env["w2_d"],
        env["b2_d"], env["vv_d"])
    ctx_d, attn_d = env["ctx_d"], env["attn_d"]

    with (
        tc.tile_pool(name="consts", bufs=1) as consts,
        tc.tile_pool(name="vpool", bufs=2) as vpool,
        tc.tile_pool(name="vtpool", bufs=2) as vtpool,
        tc.tile_pool(name="thpool", bufs=2) as thpool,
        tc.tile_pool(name="rowpool", bufs=3) as rowpool,
        tc.tile_pool(name="atpool", bufs=2) as atpool,
        tc.tile_pool(name="ctxpool", bufs=2) as ctxpool,
        tc.tile_pool(name="smallpool", bufs=2) as smallpool,
        tc.tile_pool(name="pvt_pool", bufs=pvt_bufs, space="PSUM") as pvt_pool,
        tc.tile_pool(name="ppv_pool", bufs=ppv_bufs, space="PSUM") as ppv_pool,
        tc.tile_pool(name="psc_pool", bufs=psc_bufs, space="PSUM") as psc_pool,
        tc.tile_pool(name="pctx_pool", bufs=1, space="PSUM") as pctx_pool,
    ):
        identity_f32 = consts.tile([P, P], f32)
        make_identity(nc, identity_f32)
        if mm_mode == "f32r":
            identity = consts.tile([P, P], mdt)
            nc.vector.tensor_copy(identity[:], identity_f32[:])
        else:
            identity = identity_f32

        w1_sb = consts.tile([P, N_DC, U], mdt)
        nc.sync.dma_start(w1_sb[:], mb(w1_d.rearrange("(c p) u -> p c u", p=P)))
        w2_sb = consts.tile([P, N_DC, U], mdt)
        nc.sync.dma_start(w2_sb[:], mb(w2_d.rearrange("(c p) u -> p c u", p=P)))
        v_sb = consts.tile([P, N_UB], mdt)
        nc.sync.dma_start(v_sb[:], mb(vv_d.rearrange("(c p) o -> p (c o)", p=P)))
        b1_sb = consts.tile([P, N_UB], f32)
        nc.sync.dma_start(b1_sb[:], b1_d.rearrange("(c p) -> p c", p=P))
        b2_sb = consts.tile([P, N_UB], f32)
        nc.sync.dma_start(b2_sb[:], b2_d.rearrange("(c p) -> p c", p=P))
        q_sb = consts.tile([n_ex, D], mdt)
        nc.sync.dma_start(q_sb[:], mb(q_d[:]))

        b12_sb = consts.tile([P, N_UB], f32)
        nc.vector.tensor_add(b12_sb[:], b1_sb[:], b2_sb[:])

        negshift = consts.tile([1, 1], f32)
        nc.vector.memset(negshift[:], -12.0)

        qt_sb = consts.tile([P, N_DC, n_ex], mdt)
        for dc in range(N_DC):
            pqt = pvt_pool.tile([P, n_ex], mdt, tag="pvt", name="pqt")
            nc.tensor.transpose(
                pqt[:], q_sb[:, dc * P : (dc + 1) * P], mb(identity[:n_ex, :n_ex])
            )
            nc.vector.tensor_copy(qt_sb[:, dc, :], pqt[:])

        pqb_sb = consts.tile([P, N_UB, n_ex], f32)
        for ub in range(N_UB):
            ppq = ppv_pool.tile([P, n_ex], f32, tag="ppv", name="ppq")
            for dc in range(N_DC):
                nc.tensor.matmul(
                    ppq[:],
                    w2_sb[:, dc, ub * P : (ub + 1) * P],
                    qt_sb[:, dc, :],
                    start=(dc == 0),
                    stop=(dc == N_DC - 1),
                )
            nc.vector.tensor_scalar_add(
                pqb_sb[:, ub, :], ppq[:], b12_sb[:, ub : ub + 1]
            )

        for e in [e for _ in range(reps) for e in range(n_ex)]:
            v_nat = vpool.tile([P, N_T, D], mdt)
            for t in range(N_T):
                nc.sync.dma_start(
                    v_nat[:, t, :], mb(v_d[e, t * P : (t + 1) * P, :])
                )

            vt = vtpool.tile([P, N_DC, S], mdt)
            for t in range(N_T):
                pvt = pvt_pool.tile([P, N_DC, P], mdt, tag="pvt")
                for dc in range(N_DC):
                    nc.tensor.transpose(
                        pvt[:, dc, :],
                        v_nat[:, t, dc * P : (dc + 1) * P],
                        mb(identity[:]),
                    )
                if t % evac_mod != evac_mod - 1:
                    nc.vector.tensor_copy(vt[:, :, t * P : (t + 1) * P], pvt[:])
                else:
                    nc.scalar.copy(vt[:, :, t * P : (t + 1) * P], pvt[:])

            th = thpool.tile([P, N_UB, S], mdt)
            for ub in range(N_UB):
                for sb in range(N_SB):
                    ppv = ppv_pool.tile([P, 512], f32, tag="ppv")
                    for dc in range(N_DC):
                        nc.tensor.matmul(
                            ppv[:],
                            w1_sb[:, dc, ub * P : (ub + 1) * P],
                            vt[:, dc, sb * 512 : (sb + 1) * 512],
                            start=(dc == 0),
                            stop=(dc == N_DC - 1),
                        )
                    nc.scalar.activation(
                        th[:, ub, sb * 512 : (sb + 1) * 512],
                        ppv[:],
                        AF.Tanh,
                        bias=pqb_sb[:, ub, e : e + 1],
                        scale=1.0,
                    )

            score_row = rowpool.tile([1, S], f32, tag="row", name="score_row")
            for sb in range(N_SB):
                psc = psc_pool.tile([1, 512], f32, tag="psc")
                for ub in range(N_UB):
                    nc.tensor.matmul(
                        psc[:],
                        v_sb[:, ub : ub + 1],
                        th[:, ub, sb * 512 : (sb + 1) * 512],
                        start=(ub == 0),
                        stop=(ub == N_UB - 1),
                    )
                nc.vector.tensor_copy(score_row[:, sb * 512 : (sb + 1) * 512], psc[:])

            exp_row = rowpool.tile([1, S], f32, tag="row", name="exp_row")
            sumexp = smallpool.tile([1, 1], f32, tag="sumexp")
            nc.scalar.activation(
                exp_row[:], score_row[:], AF.Exp, bias=negshift[:], scale=1.0,
                accum_out=sumexp[:],
            )
            recip = smallpool.tile([1, 1], f32, tag="recip")
            nc.vector.reciprocal(recip[:], sumexp[:])
            attn_row = rowpool.tile([1, S], f32, tag="row", name="attn_row")
            nc.gpsimd.tensor_scalar_mul(attn_row[:], exp_row[:], recip[:])
            nc.sync.dma_start(attn_d[e : e + 1, :], attn_row[:])

            pat = psc_pool.tile([P, N_T], f32, tag="psc", name="pat")
            for t in range(N_T):
                nc.tensor.transpose(
                    pat[:, t : t + 1],
                    exp_row[:, t * P : (t + 1) * P],
                    identity_f32[:1, :1],
                )
            at_sb = atpool.tile([P, N_T], mdt)
            nc.vector.tensor_copy(at_sb[:], pat[:])

            pctx = pctx_pool.tile([1, D], f32, tag="pctx")
            for t in range(N_T):
                nc.tensor.matmul(
                    pctx[:],
                    at_sb[:, t : t + 1],
                    v_nat[:, t, :],
                    start=(t == 0),
                    stop=(t == N_T - 1),
                )
            ctx_row = ctxpool.tile([1, D], f32, tag="ctx_row")
            nc.vector.tensor_scalar_mul(ctx_row[:], pctx[:], recip[:])
            nc.sync.dma_start(ctx_d[e : e + 1, :], ctx_row[:])


def kernel(query, values, W1, b1, W2, b2, V, bV):
    import ml_dtypes
    from concourse.bass_utils import run_bass_kernel_spmd

    key = ("nc", N_EX, MM_MODE)
    if key not in _CACHE:
        _CACHE[key] = build_nc(N_EX, MM_MODE)
    nc = _CACHE[key]

    query = np.asarray(query, dtype=np.float32)
    values = np.asarray(values, dtype=np.float32)
    shared = {
        "W1": np.asarray(W1, dtype=np.float32),
        "b1": np.asarray(b1, dtype=np.float32),
        "W2": np.asarray(W2, dtype=np.float32),
        "b2": np.asarray(b2, dtype=np.float32),
        "V": np.asarray(V, dtype=np.float32),
        "bV": np.asarray(bV, dtype=np.float32),
    }
    in_maps = []
    for c in range(N_CORES):
        lo, hi = c * N_EX, (c + 1) * N_EX
        m = {"query": query[lo:hi], **shared}
        if MM_MODE in ("bf16x", "bf16t"):
            vb = values[lo:hi].astype(ml_dtypes.bfloat16)
            m["values_bf16"] = vb
            if MM_MODE == "bf16t":
                m["values_bf16_T"] = np.ascontiguousarray(vb.transpose(0, 2, 1))
        else:
            m["values"] = values[lo:hi]
        in_maps.append(m)

    res = run_bass_kernel_spmd(nc, in_maps, list(range(N_CORES)))

    context = np.concatenate([res.results[c]["context"] for c in range(N_CORES)], axis=0)
    attn = np.concatenate([res.results[c]["attn"] for c in range(N_CORES)], axis=0)
    return context, attn[:, :, None]
